# revision 1
# baseline (speedup 1.0000x reference)
"""Trainium2 Bass kernel for nn_Conv2d_mvm (crossbar-quantized 3x3 conv).

The reference simulates a bit-sliced crossbar. Reductions:

1. The ADC clip [0, 511] can never bind (max per-xbar analog sum is
   128 rows * max slice digit 3 = 384), so the computation is exactly
   linear in the bit decompositions.

2. The weight reconstruction applies slice_w[0] = -2^14 to the whole
   MSB 2-bit digit, which is NOT true 2's complement: net effect the
   conv uses effective weights  w_eff = wi - 32768*[wi < 0]  with
   wi = rne(4096*w), and xi = rne(4096*x) exactly.

3. Precision slack: the harness gate is rel_err < 2e-2 and the output
   is ~95% saturated at +-8. Storing w_eff directly as fp16
   (|err| <= 8 vs acc rms ~2e9), xi as fp16(4096 x) (no integer
   rounding), and skipping the final round-to-nearest all measure
   rel err ~1.4e-3 on the real data - 14x under the gate. This
   removes the hi/lo activation split AND the separate mask matmul
   group: 9 accumulating K=64 fp16 matmuls total, x and w each DMA'd
   once (234KB/core instead of 469KB).

Implementation (8 cores, data-parallel over batch x row-blocks):
  - core c handles batch c//4, output rows 8*(c%4) .. 8*(c%4)+8
  - host pads x (zero pad=1), packs the [64, 10, 34] x-section and the
    [64, 3*3*64] (ci, kh, kw, co) weight block into one [64, 916] f32
    input per core. DMAs are split by PARTITION halves across the two
    HW-DGE rings (sync + scalar) - 32 packets per ring per tensor
    instead of 64 - with w first (its DVE chain is longer than x's
    ACT chain).
  - on device: xbuf = fp16(4096 x) (one ACT copy op);
    mneg = -32768*[w < -1/8192] and weff = fp16(4096 w + mneg) (two
    DVE ops). 9 accumulating K=64 fp16 matmuls (one per tap) into one
    PSUM bank produce acc for 270 psum columns (8 output rows x 34
    padded cols, garbage in the 2 pad columns). Post: r0 = acc/2^24
    (ACT), v0 = clip(r0, -8, 32767/4096) (one DVE tensor_scalar);
    skipping the reference's rne adds <= 1.2e-4 abs err. DMA the full
    [64, 270] f32 block out; the host slices the valid 32-col row
    segments (pure indexing).
  - The PE clock ramp is proportional to injected MAC work, so the
    warm-up dummies are K=128 fp8 matmuls (4x the MAC rate of the
    K=64 fp16 real ones). They read never-written SBUF garbage (only
    numeric garbage into a scratch PSUM bank, discarded) so they need
    no memsets/semaphores and issue as the PE's first instructions.
  - No explicit end-of-program drain/barrier: the NEFF fini block's
    own per-engine drain + token barrier covers out-DMA completion.

All value arithmetic happens on device; the host only pads, shards,
reshapes and gathers.
"""

from contextlib import ExitStack

import numpy as np

import concourse.bass as bass
import concourse.mybir as mybir
from concourse.bass_utils import run_bass_kernel_spmd

# fixed problem shape
B, C, H, W = 2, 64, 32, 32
COUT = 64
RPC = 8                    # output rows per core
SECR = RPC + 2             # padded rows per section
SECW = W + 2               # padded width
LEN = SECR * SECW          # 340
NOUT = (RPC - 1) * SECW + W  # 270 psum columns covering all valid pixels
OFFS = [dh * SECW + dw for dh in range(3) for dw in range(3)]
NW = 9 * COUT              # 576
NWH = NW // 2              # 288, per-ring weight half
NIN = LEN + NW             # 916 packed input columns
XH = LEN // 2              # 170, per-ring x half
CH = C // 2                # 32, output partition half per ring

AMAX = 32767.0 / 4096.0
AMIN = -8.0
SCL = 0.5 ** 24            # psum -> output scale
NDUM = 6                   # big (N=512) PE warm-up dummy matmuls
NDUM_SM = 3                # short trailing warm-up matmuls
NSM = 224                  # their column count

F32 = mybir.dt.float32
F16 = mybir.dt.float16
F8 = mybir.dt.float8e4

# The NEFF fini block resets every HW semaphore below the compiler's
# max-sem-num bound, ~51 per engine serially (~6.5us, dominated by the
# PE's ~127ns/write). Our program uses 7 semaphores. Packing bass's
# kernel semaphores just above walrus's internal ones and telling
# walrus the bound shrinks the reset sweep accordingly.
MAX_SEM = 64


def _patch_sem_budget():
    import concourse.bass_utils as bu
    if getattr(bu, "_sem_budget_patched", False):
        return
    bass.get_walrus_max_sem_num = lambda: MAX_SEM - 8
    orig = bu.get_walrus_args

    def patched(*a, **k):
        return [*orig(*a, **k), f"--max-sem-num={MAX_SEM}"]

    bu.get_walrus_args = patched
    bu._sem_budget_patched = True


_CACHED = None


def _build():
    _patch_sem_budget()
    nc = bass.Bass("TRN2", target_bir_lowering=False, debug=False, num_devices=8,
                   monotonic_sem_count=0)
    main = nc.m.functions[0].blocks[0]
    assert main.name == "main"
    n_preamble = len(main.instructions)

    xwin = nc.dram_tensor("xw", [C, NIN], F32, kind="ExternalInput").ap()
    yout = nc.dram_tensor("y", [COUT, NOUT], F32, kind="ExternalOutput").ap()

    with ExitStack() as ctx:
        xw2 = ctx.enter_context(nc.sbuf_tensor([C, NIN], F32))
        xbuf = ctx.enter_context(nc.sbuf_tensor([C, LEN], F16))
        mneg = ctx.enter_context(nc.sbuf_tensor([C, NW], F16))
        weff = ctx.enter_context(nc.sbuf_tensor([C, NW], F16))
        r0 = ctx.enter_context(nc.sbuf_tensor([COUT, NOUT], F32))
        v0 = ctx.enter_context(nc.sbuf_tensor([COUT, NOUT], F32))
        scr = ctx.enter_context(nc.sbuf_tensor([1, 8], F32))
        wdum = ctx.enter_context(nc.sbuf_tensor([2 * C, C], F16))
        mdum = ctx.enter_context(nc.sbuf_tensor([2 * C, 512], F16))
        ps = ctx.enter_context(nc.psum_tensor([COUT, NOUT], F32))
        psd = ctx.enter_context(nc.psum_tensor([COUT, 512], F32))
        s_a = ctx.enter_context(nc.semaphore())
        s_b = ctx.enter_context(nc.semaphore())
        s_w2 = ctx.enter_context(nc.semaphore())
        s_act = ctx.enter_context(nc.semaphore())
        s_dve = ctx.enter_context(nc.semaphore())
        s_p = ctx.enter_context(nc.semaphore())

        AL = mybir.AluOpType
        CP = mybir.ActivationFunctionType.Copy

        # ---- input DMAs: w and x each split three ways across the
        # sync/scalar/gpsimd queues (~78KB per queue), w thirds first
        # (its dependent chain is the longer one); per-queue w
        # semaphores so the mneg chunks can chase the landings ----
        WT, XT = NW // 3, 114  # w third = 192 cols, x thirds 114/114/112
        nc.sync.dma_start(xw2[:, LEN:LEN + WT], xwin[:, LEN:LEN + WT]).then_inc(s_b, 16)
        nc.scalar.dma_start(xw2[:, LEN + WT:LEN + 2 * WT], xwin[:, LEN + WT:LEN + 2 * WT]).then_inc(s_w2, 16)
        nc.gpsimd.dma_start(xw2[:, LEN + 2 * WT:NIN], xwin[:, LEN + 2 * WT:NIN]).then_inc(s_p, 16)
        nc.sync.dma_start(xw2[:, 0:XT], xwin[:, 0:XT]).then_inc(s_a, 16)
        nc.scalar.dma_start(xw2[:, XT:2 * XT], xwin[:, XT:2 * XT]).then_inc(s_a, 16)
        nc.gpsimd.dma_start(xw2[:, 2 * XT:LEN], xwin[:, 2 * XT:LEN]).then_inc(s_a, 16)

        # ---- PE: warm-up group first (garbage-input, K=128).  The PE
        # clock ramp decays within ~1us of idle, so after the big block
        # a tail of short dummies keeps the array hot until the real
        # matmuls unblock (overshoot granularity ~250ns). ----
        for i in range(NDUM):
            nc.tensor.matmul(psd[:], wdum[:, 0:COUT], mdum[:], start=(i == 0), stop=False)
        for i in range(NDUM_SM):
            nc.tensor.matmul(psd[:, 0:NSM], wdum[:, 0:COUT], mdum[:, 0:NSM],
                             start=False, stop=(i == NDUM_SM - 1))

        # ---- ACT: table preload (garbage input, output unused), x quant ----
        nc.scalar.activation(scr[:], scr[:], CP, bias=0.0, scale=0.0).then_inc(s_act, 1)
        nc.scalar.wait_ge(s_a, 48)
        # xbuf = fp16(4096*x)
        nc.scalar.activation(xbuf[:], xw2[:, 0:LEN], CP, bias=0.0, scale=4096.0).then_inc(s_act, 1)

        # ---- DVE weight chain: mneg = -32768 * [wi < 0] (from raw w:
        # wi = rne(4096*w) < 0 <=> w < -1/8192, ties round to -0)
        # chunk-by-chunk as each queue's w third lands, then one
        # weff = fp16(4096*w + mneg) over the lot ----
        nc.vector.wait_ge(s_w2, 16)
        nc.vector.tensor_scalar(mneg[:, WT:2 * WT], xw2[:, LEN + WT:LEN + 2 * WT], -1.0 / 8192.0, -32768.0, AL.is_lt, AL.mult)
        nc.vector.wait_ge(s_b, 16)
        nc.vector.tensor_scalar(mneg[:, 0:WT], xw2[:, LEN:LEN + WT], -1.0 / 8192.0, -32768.0, AL.is_lt, AL.mult)
        nc.vector.wait_ge(s_p, 16)
        nc.vector.tensor_scalar(mneg[:, 2 * WT:NW], xw2[:, LEN + 2 * WT:NIN], -1.0 / 8192.0, -32768.0, AL.is_lt, AL.mult).then_inc(s_dve, 1)
        nc.vector.wait_ge(s_dve, 1)
        nc.vector.scalar_tensor_tensor(weff[:], xw2[:, LEN:NIN], 4096.0, mneg[:], AL.mult, AL.add).then_inc(s_dve, 1)

        # ---- PE: the real conv ----
        nc.tensor.wait_ge(s_act, 2)
        nc.tensor.wait_ge(s_dve, 2)
        for d in range(9):
            mm = nc.tensor.matmul(
                ps[:],
                weff[:, d * COUT:(d + 1) * COUT],
                xbuf[:, OFFS[d]:OFFS[d] + NOUT],
                start=(d == 0),
                stop=(d == 8),
            )
        mm.then_inc(s_act, 1)

        # ---- post, pipelined in column halves: scale on ACT, clip on
        # DVE, out-DMA per half on its own ring (fini drains cover
        # completion) ----
        NH = 136
        nc.scalar.wait_ge(s_act, 3)
        nc.scalar.activation(r0[:, 0:NH], ps[:, 0:NH], CP, bias=0.0, scale=SCL).then_inc(s_act, 1)
        nc.scalar.activation(r0[:, NH:NOUT], ps[:, NH:NOUT], CP, bias=0.0, scale=SCL).then_inc(s_act, 1)
        nc.vector.wait_ge(s_act, 4)
        nc.vector.tensor_scalar(v0[:, 0:NH], r0[:, 0:NH], AMAX, AMIN, AL.min, AL.max).then_inc(s_dve, 1)
        nc.vector.wait_ge(s_act, 5)
        nc.vector.tensor_scalar(v0[:, NH:NOUT], r0[:, NH:NOUT], AMAX, AMIN, AL.min, AL.max).then_inc(s_dve, 1)
        nc.sync.wait_ge(s_dve, 3)
        nc.sync.dma_start(yout[:, 0:NH], v0[:, 0:NH]).then_inc(s_a, 16)
        nc.scalar.wait_ge(s_dve, 4)
        nc.scalar.dma_start(yout[:, NH:NOUT], v0[:, NH:NOUT]).then_inc(s_a, 16)

    # Strip the framework const-AP memsets and the post-init all-engine
    # barrier (they are unused here; HW semaphores are zero at NEFF load
    # and re-zeroed by the NEFF epilogue). Only the construction-time
    # preamble prefix is touched.
    insts = main.instructions
    pre = [
        ins for ins in insts[:n_preamble]
        if type(ins).__name__ not in (
            "InstMemset", "InstDrain", "InstEventSemaphore", "InstRegisterMove")
    ]
    main.instructions = pre + insts[n_preamble:]

    return nc


def _get_nc():
    global _CACHED
    if _CACHED is None:
        _CACHED = _build()
    return _CACHED


def _shard_inputs(x, weight):
    xpad = np.pad(np.ascontiguousarray(x, dtype=np.float32),
                  ((0, 0), (0, 0), (1, 1), (1, 1)))
    wre = np.asarray(weight, dtype=np.float32).transpose(1, 2, 3, 0).reshape(C, NW)
    in_maps = []
    for c in range(8):
        b, q = divmod(c, 4)
        sec = xpad[b, :, RPC * q:RPC * q + SECR, :].reshape(C, LEN)
        xw = np.concatenate([sec, wre], axis=1)
        in_maps.append({"xw": np.ascontiguousarray(xw)})
    return in_maps


def kernel(x, weight):
    nc = _get_nc()
    in_maps = _shard_inputs(x, weight)
    res = run_bass_kernel_spmd(nc, in_maps, core_ids=list(range(8)))
    out = np.empty((B, COUT, H, W), dtype=np.float32)
    for c in range(8):
        b, q = divmod(c, 4)
        y = res.results[c]["y"]
        for r in range(RPC):
            out[b, :, RPC * q + r, :] = y[:, r * SECW:r * SECW + W]
    return out



# revision 11
# speedup vs baseline: 1.0210x; 1.0210x over previous
"""Trainium2 Bass kernel for nn_Conv2d_mvm (crossbar-quantized 3x3 conv).

The reference simulates a bit-sliced crossbar. Reductions:

1. The ADC clip [0, 511] can never bind (max per-xbar analog sum is
   128 rows * max slice digit 3 = 384), so the computation is exactly
   linear in the bit decompositions.

2. The weight reconstruction applies slice_w[0] = -2^14 to the whole
   MSB 2-bit digit, which is NOT true 2's complement: net effect the
   conv uses effective weights  w_eff = wi - 32768*[wi < 0]  with
   wi = rne(4096*w), and xi = rne(4096*x) exactly.

3. Precision slack (gate is rel_err < 2e-2; measured ~1.4e-3): skip
   integer rounding and fold the final /2^24 into the quant scales --
   fp16 is scale-invariant under powers of two, so
     xbuf = fp16(x)            (== fp16(4096 x) / 4096 exactly)
     weff = fp16(w - 8*[w < -1/8192])
   makes PSUM hold the final pre-clip value directly: no post scale
   pass, the DVE clips straight out of PSUM.

Implementation (8 cores, data-parallel over batch x row-blocks):
  - core c handles batch c//4, output rows 8*(c%4) .. 8*(c%4)+8.
  - K=128 tap packing: the host packs the [64, 10, 34] padded x
    section twice -- partitions 0-63 as-is, partitions 64-127 the
    same flat buffer shifted 35 columns (one row + one col). Taps
    with OFFS delta 35 then share one K=128 matmul (same rhs window,
    top half hits offset o, bottom o+35): pairs (0,4), (1,5), (3,7)
    are 3 K=128 matmuls; taps 2, 6, 8 are top-half K=64 matmuls.
    6 serial matmul slots instead of 9. (A 5-slot variant pairing
    taps 6/7 as concurrent (0,0)/(64,0) row-tiles compiled but died
    at runtime -- base-partition-64 K=64 matmuls are not usable.)
  - weights are packed [128, 384] f32 alongside: col blocks 0-2 hold
    the pairs stacked top/bottom, blocks 3-5 taps 2, 6, 8 on top
    with zeros below.
  - input DMAs: one [64, 660] f32 DMA per partition half, issued by
    SP (top) and GpSimd (bottom) so the ACT engine's first op is its
    activation-table preload (1.3us) with zero queue delay.
  - DVE: mneg = -8*[w < -1/8192] (fp32 w -- the threshold must be
    evaluated pre-fp16 or ~5 mask bits flip), weff = fp16(w + mneg),
    then after the matmuls clip v0 = min(max(psum, -8), 32767/4096)
    read directly from PSUM, in column halves so the out-DMAs start
    per half on SP/ACT.
  - no PE warm-up dummies: the HAM clock ramp needs ~3.4us of
    sustained PE busy which the shortened front-end no longer
    provides; the real matmuls run at 1.2GHz either way.
  - No explicit end-of-program drain/barrier: the NEFF fini block's
    own per-engine drain + token barrier covers out-DMA completion.

All value arithmetic happens on device; the host only pads, shards,
reshapes and gathers.
"""

from contextlib import ExitStack

import numpy as np

import concourse.bass as bass
import concourse.mybir as mybir
from concourse.bass_utils import run_bass_kernel_spmd

# fixed problem shape
B, C, H, W = 2, 64, 32, 32
COUT = 64
RPC = 8                    # output rows per core
SECR = RPC + 2             # padded rows per section
SECW = W + 2               # padded width
LEN = SECR * SECW          # 340
NOUT = (RPC - 1) * SECW + W  # 270 psum columns covering all valid pixels
NW = 6 * COUT              # 384 packed weight cols (3 pairs + 3 top-half singles)
NIN = LEN + NW             # 660 packed input columns per partition
NH = 136                   # out-DMA column split

AMAX = 32767.0 / 4096.0
AMIN = -8.0
THR = -1.0 / 8192.0        # wi = rne(4096 w) < 0  <=>  w < -1/8192

F32 = mybir.dt.float32
F16 = mybir.dt.float16

# The NEFF fini block resets every HW semaphore below the compiler's
# max-sem-num bound. Packing bass's kernel semaphores just above
# walrus's internal ones and telling walrus the bound keeps the
# program's semaphore footprint minimal.
MAX_SEM = 64


def _patch_sem_budget():
    import concourse.bass_utils as bu
    if getattr(bu, "_sem_budget_patched", False):
        return
    bass.get_walrus_max_sem_num = lambda: MAX_SEM - 8
    orig = bu.get_walrus_args

    def patched(*a, **k):
        return [*orig(*a, **k), f"--max-sem-num={MAX_SEM}"]

    bu.get_walrus_args = patched
    bu._sem_budget_patched = True


_CACHED = None


def _build():
    _patch_sem_budget()
    nc = bass.Bass("TRN2", target_bir_lowering=False, debug=False, num_devices=8,
                   monotonic_sem_count=0)
    main = nc.m.functions[0].blocks[0]
    assert main.name == "main"
    n_preamble = len(main.instructions)

    xwin = nc.dram_tensor("xw", [2 * C, NIN], F32, kind="ExternalInput").ap()
    yout = nc.dram_tensor("y", [COUT, NOUT], F32, kind="ExternalOutput").ap()

    with ExitStack() as ctx:
        xw2 = ctx.enter_context(nc.sbuf_tensor([2 * C, NIN], F32))
        xbuf = ctx.enter_context(nc.sbuf_tensor([2 * C, LEN], F16))
        mneg = ctx.enter_context(nc.sbuf_tensor([2 * C, NW], F16))
        weff = ctx.enter_context(nc.sbuf_tensor([2 * C, NW], F16))
        v0 = ctx.enter_context(nc.sbuf_tensor([COUT, NOUT], F32))
        scr = ctx.enter_context(nc.sbuf_tensor([1, 8], F32))
        ps = ctx.enter_context(nc.psum_tensor([COUT, NOUT], F32))
        s_in = ctx.enter_context(nc.semaphore())
        s_act = ctx.enter_context(nc.semaphore())
        s_dve = ctx.enter_context(nc.semaphore())
        s_mm = ctx.enter_context(nc.semaphore())
        s_clip = ctx.enter_context(nc.semaphore())

        AL = mybir.AluOpType
        CP = mybir.ActivationFunctionType.Copy

        # ---- input DMAs: one per partition half, off the ACT engine ----
        nc.sync.dma_start(xw2[0:C, :], xwin[0:C, :]).then_inc(s_in, 16)
        nc.gpsimd.dma_start(xw2[C:2 * C, :], xwin[C:2 * C, :]).then_inc(s_in, 16)

        # ---- ACT: table preload first (garbage input, output unused),
        # then the x quant once both input halves have landed ----
        nc.scalar.activation(scr[:], scr[:], CP, bias=0.0, scale=0.0).then_inc(s_act, 1)
        nc.scalar.wait_ge(s_in, 32)
        nc.scalar.activation(xbuf[:], xw2[:, 0:LEN], CP, bias=0.0, scale=1.0).then_inc(s_act, 1)

        # ---- DVE weight chain ----
        nc.vector.wait_ge(s_in, 32)
        nc.vector.tensor_scalar(mneg[:], xw2[:, LEN:NIN], THR, -8.0, AL.is_lt, AL.mult)
        nc.vector.scalar_tensor_tensor(weff[:], xw2[:, LEN:NIN], 1.0, mneg[:], AL.mult, AL.add).then_inc(s_dve, 1)

        # ---- PE: 6 matmul slots. The bottom x copy is the top copy
        # shifted 35 flat columns, so taps (0,4), (1,5), (3,7) each
        # share one K=128 matmul; taps 2, 6, 8 are top-half K=64. ----
        nc.tensor.wait_ge(s_dve, 1)
        nc.tensor.wait_ge(s_act, 2)
        for i, o in enumerate((0, 1, 34)):       # pairs (0,4), (1,5), (3,7)
            nc.tensor.matmul(
                ps[:],
                weff[:, i * COUT:(i + 1) * COUT],
                xbuf[:, o:o + NOUT],
                start=(i == 0),
                stop=False,
            )
        for i, o in enumerate((2, 68)):          # taps 2, 6
            nc.tensor.matmul(ps[:], weff[0:C, (3 + i) * COUT:(4 + i) * COUT],
                             xbuf[0:C, o:o + NOUT], start=False, stop=False)
        mm = nc.tensor.matmul(ps[:], weff[0:C, 5 * COUT:6 * COUT],
                              xbuf[0:C, 70:70 + NOUT], start=False, stop=True)
        mm.then_inc(s_mm, 1)

        # ---- DVE post: clip straight out of PSUM, in column halves;
        # out-DMA per half on SP/ACT (fini drains cover completion) ----
        nc.vector.wait_ge(s_mm, 1)
        nc.vector.tensor_scalar(v0[:, 0:NH], ps[:, 0:NH], AMAX, AMIN, AL.min, AL.max).then_inc(s_clip, 1)
        nc.vector.tensor_scalar(v0[:, NH:NOUT], ps[:, NH:NOUT], AMAX, AMIN, AL.min, AL.max).then_inc(s_clip, 1)
        nc.sync.wait_ge(s_clip, 1)
        nc.sync.dma_start(yout[:, 0:NH], v0[:, 0:NH]).then_inc(s_in, 16)
        nc.scalar.wait_ge(s_clip, 2)
        nc.scalar.dma_start(yout[:, NH:NOUT], v0[:, NH:NOUT]).then_inc(s_in, 16)

    # Strip the framework const-AP memsets and the post-init all-engine
    # barrier (they are unused here; HW semaphores are zero at NEFF load
    # and re-zeroed by the NEFF epilogue). Only the construction-time
    # preamble prefix is touched.
    insts = main.instructions
    pre = [
        ins for ins in insts[:n_preamble]
        if type(ins).__name__ not in (
            "InstMemset", "InstDrain", "InstEventSemaphore", "InstRegisterMove")
    ]
    main.instructions = pre + insts[n_preamble:]

    return nc


def _get_nc():
    global _CACHED
    if _CACHED is None:
        _CACHED = _build()
    return _CACHED


def _shard_inputs(x, weight):
    xpad = np.pad(np.ascontiguousarray(x, dtype=np.float32),
                  ((0, 0), (0, 0), (1, 1), (1, 1)))
    wre = np.asarray(weight, dtype=np.float32).transpose(1, 2, 3, 0)  # [ci, kh, kw, co]
    wtap = [wre[:, t // 3, t % 3, :] for t in range(9)]               # each [C, COUT]
    w_top = np.concatenate([wtap[0], wtap[1], wtap[3], wtap[2], wtap[6], wtap[8]], axis=1)
    w_bot = np.concatenate([wtap[4], wtap[5], wtap[7],
                            np.zeros((C, 3 * COUT), np.float32)], axis=1)
    in_maps = []
    for c in range(8):
        b, q = divmod(c, 4)
        top = xpad[b, :, RPC * q:RPC * q + SECR, :].reshape(C, LEN)
        bot = np.zeros((C, LEN), np.float32)
        bot[:, :LEN - 35] = top[:, 35:]
        xw = np.empty((2 * C, NIN), np.float32)
        xw[0:C, 0:LEN] = top
        xw[C:2 * C, 0:LEN] = bot
        xw[0:C, LEN:NIN] = w_top
        xw[C:2 * C, LEN:NIN] = w_bot
        in_maps.append({"xw": np.ascontiguousarray(xw)})
    return in_maps


def kernel(x, weight):
    nc = _get_nc()
    in_maps = _shard_inputs(x, weight)
    res = run_bass_kernel_spmd(nc, in_maps, core_ids=list(range(8)))
    out = np.empty((B, COUT, H, W), dtype=np.float32)
    for c in range(8):
        b, q = divmod(c, 4)
        y = res.results[c]["y"]
        for r in range(RPC):
            out[b, :, RPC * q + r, :] = y[:, r * SECW:r * SECW + W]
    return out


# revision 13
# speedup vs baseline: 1.0965x; 1.0739x over previous
"""Trainium2 Bass kernel for nn_Conv2d_mvm (crossbar-quantized 3x3 conv).

The reference simulates a bit-sliced crossbar. Reductions:

1. The ADC clip [0, 511] can never bind (max per-xbar analog sum is
   128 rows * max slice digit 3 = 384), so the computation is exactly
   linear in the bit decompositions.

2. The weight reconstruction applies slice_w[0] = -2^14 to the whole
   MSB 2-bit digit, which is NOT true 2's complement: net effect the
   conv uses effective weights  w_eff = wi - 32768*[wi < 0]  with
   wi = rne(4096*w), and xi = rne(4096*x) exactly.

3. Precision slack (gate is rel_err < 2e-2; measured ~1.4e-3): skip
   integer rounding and fold the final /2^24 into the quant scales --
   fp16 is scale-invariant under powers of two, so
     xbuf = fp16(x)            (== fp16(4096 x) / 4096 exactly)
     weff = fp16(w - 8*[w < -1/8192])
   makes PSUM hold the final pre-clip value directly: no post scale
   pass; the DVE clips straight out of PSUM. fp16(x) is produced by
   the SWDGE cast-during-DMA, so the ACT engine runs no compute at
   all (no activation-table load on the critical path).

Implementation (8 cores, data-parallel over batch x row-blocks):
  - core c handles batch c//4, output rows 8*(c%4) .. 8*(c%4)+8.
  - K=128 tap packing: x lands twice -- partitions 0-63 the padded
    [10, 34] section flat, partitions 64-127 the same flat buffer
    shifted 35 columns (one row + one col). Taps with OFFS delta 35
    share one K=128 matmul (top half hits offset o, bottom o+35):
    pairs (0,4), (1,5), (3,7); taps 2, 6, 8 stay top-half K=64.
    6 matmul slots instead of 9, singles first (they only need the
    top x half + top-half weights, which land earliest).
    (A 5-slot variant with taps 6/7 as concurrent (0,0)/(64,0)
    row-tiles compiled but died at runtime -- base-partition-64
    K=64 matmuls are not usable.)
  - weights [128, 384] f32: col blocks 0-2 hold the pairs stacked
    top/bottom, blocks 3-5 taps 2, 6, 8 in the top half only.
  - DMA split: SP carries w_top [64,384], ACT w_bot [64,192] (both
    HWDGE, issued immediately), GpSimd the two x cast-DMAs
    (f32 DRAM -> f16 SBUF, top then bottom). Separate semaphores per
    piece so each consumer waits only for what it reads.
  - DVE: mneg = -8*[w < -1/8192] (fp32 w -- the threshold must be
    evaluated pre-fp16 or ~5 mask bits flip), weff = fp16(w + mneg),
    singles chunk first (top w only), pairs chunk second; after the
    matmuls clip v0 = min(max(psum, -8), 32767/4096) read directly
    from PSUM, in column halves so the out-DMAs start per half on
    SP/ACT (fini drains cover completion).
  - no PE warm-up dummies: the HAM clock ramp needs ~3.4us of
    sustained PE busy which this short front-end cannot provide; the
    real matmuls run at 1.2GHz either way.

All value arithmetic happens on device; the host only pads, shards,
reshapes and gathers.
"""

from contextlib import ExitStack

import numpy as np

import concourse.bass as bass
import concourse.mybir as mybir
from concourse.bass_utils import run_bass_kernel_spmd

# fixed problem shape
B, C, H, W = 2, 64, 32, 32
COUT = 64
RPC = 8                    # output rows per core
SECR = RPC + 2             # padded rows per section
SECW = W + 2               # padded width
LEN = SECR * SECW          # 340
NOUT = (RPC - 1) * SECW + W  # 270 psum columns covering all valid pixels
NW = 6 * COUT              # 384 packed weight cols (3 pairs + 3 singles)
NWB = 3 * COUT             # 192 bottom-half weight cols (pairs only)
NIN = LEN + NW             # 724 packed input columns per partition
NH = 136                   # out-DMA column split

AMAX = 32767.0 / 4096.0
AMIN = -8.0
THR = -1.0 / 8192.0        # wi = rne(4096 w) < 0  <=>  w < -1/8192

F32 = mybir.dt.float32
F16 = mybir.dt.float16

# The NEFF fini block resets every HW semaphore below the compiler's
# max-sem-num bound. Packing bass's kernel semaphores just above
# walrus's internal ones and telling walrus the bound keeps the
# program's semaphore footprint minimal.
MAX_SEM = 64


def _patch_sem_budget():
    import concourse.bass_utils as bu
    if getattr(bu, "_sem_budget_patched", False):
        return
    bass.get_walrus_max_sem_num = lambda: MAX_SEM - 8
    orig = bu.get_walrus_args

    def patched(*a, **k):
        return [*orig(*a, **k), f"--max-sem-num={MAX_SEM}"]

    bu.get_walrus_args = patched
    bu._sem_budget_patched = True


_CACHED = None


def _build():
    _patch_sem_budget()
    nc = bass.Bass("TRN2", target_bir_lowering=False, debug=False, num_devices=8,
                   monotonic_sem_count=0)
    main = nc.m.functions[0].blocks[0]
    assert main.name == "main"
    n_preamble = len(main.instructions)

    xwin = nc.dram_tensor("xw", [2 * C, NIN], F32, kind="ExternalInput").ap()
    yout = nc.dram_tensor("y", [COUT, NOUT], F32, kind="ExternalOutput").ap()

    with ExitStack() as ctx:
        wraw = ctx.enter_context(nc.sbuf_tensor([2 * C, NW], F32))
        xbuf = ctx.enter_context(nc.sbuf_tensor([2 * C, LEN], F16))
        mneg = ctx.enter_context(nc.sbuf_tensor([2 * C, NW], F16))
        weff = ctx.enter_context(nc.sbuf_tensor([2 * C, NW], F16))
        v0 = ctx.enter_context(nc.sbuf_tensor([COUT, NOUT], F32))
        ps = ctx.enter_context(nc.psum_tensor([COUT, NOUT], F32))
        s_wt = ctx.enter_context(nc.semaphore())
        s_wb = ctx.enter_context(nc.semaphore())
        s_xt = ctx.enter_context(nc.semaphore())
        s_xb = ctx.enter_context(nc.semaphore())
        s_dve = ctx.enter_context(nc.semaphore())
        s_mm = ctx.enter_context(nc.semaphore())
        s_clip = ctx.enter_context(nc.semaphore())

        AL = mybir.AluOpType

        # ---- input DMAs: w halves on the two HWDGE queues, x via
        # SWDGE cast-DMA (f32 -> f16), top half first ----
        nc.sync.dma_start(wraw[0:C, :], xwin[0:C, LEN:NIN]).then_inc(s_wt, 16)
        nc.scalar.dma_start(wraw[C:2 * C, 0:NWB], xwin[C:2 * C, LEN:LEN + NWB]).then_inc(s_wb, 16)
        nc.gpsimd.dma_start(xbuf[0:C, :], xwin[0:C, 0:LEN]).then_inc(s_xt, 16)
        nc.gpsimd.dma_start(xbuf[C:2 * C, :], xwin[C:2 * C, 0:LEN]).then_inc(s_xb, 16)

        # ---- DVE weight chain: singles chunk (top-only) first ----
        nc.vector.wait_ge(s_wt, 16)
        nc.vector.tensor_scalar(mneg[0:C, NWB:NW], wraw[0:C, NWB:NW], THR, -8.0, AL.is_lt, AL.mult)
        nc.vector.scalar_tensor_tensor(weff[0:C, NWB:NW], wraw[0:C, NWB:NW], 1.0, mneg[0:C, NWB:NW], AL.mult, AL.add).then_inc(s_dve, 1)
        nc.vector.wait_ge(s_wb, 16)
        nc.vector.tensor_scalar(mneg[:, 0:NWB], wraw[:, 0:NWB], THR, -8.0, AL.is_lt, AL.mult)
        nc.vector.scalar_tensor_tensor(weff[:, 0:NWB], wraw[:, 0:NWB], 1.0, mneg[:, 0:NWB], AL.mult, AL.add).then_inc(s_dve, 1)

        # ---- PE: 6 matmul slots, singles first ----
        nc.tensor.wait_ge(s_dve, 1)
        nc.tensor.wait_ge(s_xt, 16)
        for i, o in enumerate((2, 68, 70)):      # taps 2, 6, 8
            nc.tensor.matmul(ps[:], weff[0:C, (3 + i) * COUT:(4 + i) * COUT],
                             xbuf[0:C, o:o + NOUT], start=(i == 0), stop=False)
        nc.tensor.wait_ge(s_dve, 2)
        nc.tensor.wait_ge(s_xb, 16)
        for i, o in enumerate((0, 1, 34)):       # pairs (0,4), (1,5), (3,7)
            mm = nc.tensor.matmul(ps[:], weff[:, i * COUT:(i + 1) * COUT],
                                  xbuf[:, o:o + NOUT], start=False, stop=(i == 2))
        mm.then_inc(s_mm, 1)

        # ---- DVE post: clip straight out of PSUM, in column halves;
        # out-DMA per half on SP/ACT (fini drains cover completion) ----
        nc.vector.wait_ge(s_mm, 1)
        nc.vector.tensor_scalar(v0[:, 0:NH], ps[:, 0:NH], AMAX, AMIN, AL.min, AL.max).then_inc(s_clip, 1)
        nc.vector.tensor_scalar(v0[:, NH:NOUT], ps[:, NH:NOUT], AMAX, AMIN, AL.min, AL.max).then_inc(s_clip, 1)
        nc.sync.wait_ge(s_clip, 1)
        nc.sync.dma_start(yout[:, 0:NH], v0[:, 0:NH]).then_inc(s_wt, 16)
        nc.scalar.wait_ge(s_clip, 2)
        nc.scalar.dma_start(yout[:, NH:NOUT], v0[:, NH:NOUT]).then_inc(s_wb, 16)

    # Strip the framework const-AP memsets and the post-init all-engine
    # barrier (they are unused here; HW semaphores are zero at NEFF load
    # and re-zeroed by the NEFF epilogue). Only the construction-time
    # preamble prefix is touched.
    insts = main.instructions
    pre = [
        ins for ins in insts[:n_preamble]
        if type(ins).__name__ not in (
            "InstMemset", "InstDrain", "InstEventSemaphore", "InstRegisterMove")
    ]
    main.instructions = pre + insts[n_preamble:]

    return nc


def _get_nc():
    global _CACHED
    if _CACHED is None:
        _CACHED = _build()
    return _CACHED


def _shard_inputs(x, weight):
    xpad = np.pad(np.ascontiguousarray(x, dtype=np.float32),
                  ((0, 0), (0, 0), (1, 1), (1, 1)))
    wre = np.asarray(weight, dtype=np.float32).transpose(1, 2, 3, 0)  # [ci, kh, kw, co]
    wtap = [wre[:, t // 3, t % 3, :] for t in range(9)]               # each [C, COUT]
    w_top = np.concatenate([wtap[0], wtap[1], wtap[3], wtap[2], wtap[6], wtap[8]], axis=1)
    w_bot = np.concatenate([wtap[4], wtap[5], wtap[7]], axis=1)
    in_maps = []
    for c in range(8):
        b, q = divmod(c, 4)
        top = xpad[b, :, RPC * q:RPC * q + SECR, :].reshape(C, LEN)
        xw = np.zeros((2 * C, NIN), np.float32)
        xw[0:C, 0:LEN] = top
        xw[C:2 * C, 0:LEN - 35] = top[:, 35:]
        xw[0:C, LEN:NIN] = w_top
        xw[C:2 * C, LEN:LEN + NWB] = w_bot
        in_maps.append({"xw": np.ascontiguousarray(xw)})
    return in_maps


def kernel(x, weight):
    nc = _get_nc()
    in_maps = _shard_inputs(x, weight)
    res = run_bass_kernel_spmd(nc, in_maps, core_ids=list(range(8)))
    out = np.empty((B, COUT, H, W), dtype=np.float32)
    for c in range(8):
        b, q = divmod(c, 4)
        y = res.results[c]["y"]
        for r in range(RPC):
            out[b, :, RPC * q + r, :] = y[:, r * SECW:r * SECW + W]
    return out


# revision 16
# speedup vs baseline: 1.1161x; 1.0179x over previous
"""Trainium2 Bass kernel for nn_Conv2d_mvm (crossbar-quantized 3x3 conv).

The reference simulates a bit-sliced crossbar. Reductions:

1. The ADC clip [0, 511] can never bind (max per-xbar analog sum is
   128 rows * max slice digit 3 = 384), so the computation is exactly
   linear in the bit decompositions.

2. The weight reconstruction applies slice_w[0] = -2^14 to the whole
   MSB 2-bit digit, which is NOT true 2's complement: net effect the
   conv uses effective weights  w_eff = wi - 32768*[wi < 0]  with
   wi = rne(4096*w), and xi = rne(4096*x) exactly.

3. Precision slack (gate is rel_err < 2e-2; measured ~1.4e-3): skip
   integer rounding and fold the final /2^24 into the quant scales --
   fp16 is scale-invariant under powers of two, so
     xbuf = fp16(x)            (== fp16(4096 x) / 4096 exactly)
     weff = fp16(w - 8*[w < -1/8192])
   makes PSUM hold the final pre-clip value directly: no post scale
   pass; the DVE clips straight out of PSUM. fp16(x) is produced by
   the SWDGE cast-during-DMA, so the ACT engine runs no compute at
   all (no activation-table load on the critical path).

Implementation (8 cores, data-parallel over batch x row-blocks):
  - core c handles batch c//4, output rows 8*(c%4) .. 8*(c%4)+8.
  - K=128 tap packing: x lands twice -- partitions 0-63 the padded
    [10, 34] section flat, partitions 64-127 the same flat buffer
    shifted 35 columns (one row + one col). Taps with OFFS delta 35
    share one K=128 matmul (top half hits offset o, bottom o+35):
    pairs (0,4), (1,5), (3,7); taps 2, 6, 8 stay top-half K=64.
    6 matmul slots instead of 9, singles first (they only need the
    top x half + top-half weights, which land earliest).
    (A 5-slot variant with taps 6/7 as concurrent (0,0)/(64,0)
    row-tiles compiled but died at runtime -- base-partition-64
    K=64 matmuls are not usable.)
  - weights [128, 384] f32: col blocks 0-2 hold the pairs stacked
    top/bottom, blocks 3-5 taps 2, 6, 8 in the top half only.
  - DMA split: SP carries w_top [64,384], ACT w_bot [64,192] (both
    HWDGE, issued immediately), GpSimd the two x cast-DMAs
    (f32 DRAM -> f16 SBUF, top then bottom). Separate semaphores per
    piece so each consumer waits only for what it reads.
  - DVE: mneg = -8*[w < -1/8192] (fp32 w -- the threshold must be
    evaluated pre-fp16 or ~5 mask bits flip), weff = fp16(w + mneg),
    singles chunk first (top w only), pairs chunk second; after the
    matmuls clip v0 = min(max(psum, -8), 32767/4096) read directly
    from PSUM, in column halves so the out-DMAs start per half on
    SP/ACT (fini drains cover completion).
  - PE warm-up: 8 garbage-input K=128 N=512 matmuls issue as the
    PE's first instructions (~3.4us at the cold 1.2GHz clock), so the
    HAM activity window un-throttles the PE clock to 2.4GHz right as
    the real matmuls (which start ~3.3us after main-block entry)
    begin. They read never-written SBUF into a scratch PSUM bank --
    no memsets or semaphores needed.

All value arithmetic happens on device; the host only pads, shards,
reshapes and gathers.
"""

from contextlib import ExitStack

import numpy as np

import concourse.bass as bass
import concourse.mybir as mybir
from concourse.bass_utils import run_bass_kernel_spmd

# fixed problem shape
B, C, H, W = 2, 64, 32, 32
COUT = 64
RPC = 8                    # output rows per core
SECR = RPC + 2             # padded rows per section
SECW = W + 2               # padded width
LEN = SECR * SECW          # 340
NOUT = (RPC - 1) * SECW + W  # 270 psum columns covering all valid pixels
NW = 6 * COUT              # 384 packed weight cols (3 pairs + 3 singles)
NWB = 3 * COUT             # 192 bottom-half weight cols (pairs only)
NIN = LEN + NW             # 724 packed input columns per partition
NH = 136                   # out-DMA column split

AMAX = 32767.0 / 4096.0
AMIN = -8.0
THR = -1.0 / 8192.0        # wi = rne(4096 w) < 0  <=>  w < -1/8192

F32 = mybir.dt.float32
F16 = mybir.dt.float16

# The NEFF fini block resets every HW semaphore below the compiler's
# max-sem-num bound. Packing bass's kernel semaphores just above
# walrus's internal ones and telling walrus the bound keeps the
# program's semaphore footprint minimal.
MAX_SEM = 64


def _patch_sem_budget():
    import concourse.bass_utils as bu
    if getattr(bu, "_sem_budget_patched", False):
        return
    bass.get_walrus_max_sem_num = lambda: MAX_SEM - 8
    orig = bu.get_walrus_args

    def patched(*a, **k):
        return [*orig(*a, **k), f"--max-sem-num={MAX_SEM}"]

    bu.get_walrus_args = patched
    bu._sem_budget_patched = True


_CACHED = None


def _build():
    _patch_sem_budget()
    nc = bass.Bass("TRN2", target_bir_lowering=False, debug=False, num_devices=8,
                   monotonic_sem_count=0)
    main = nc.m.functions[0].blocks[0]
    assert main.name == "main"
    n_preamble = len(main.instructions)

    xwin = nc.dram_tensor("xw", [2 * C, NIN], F32, kind="ExternalInput").ap()
    yout = nc.dram_tensor("y", [COUT, NOUT], F32, kind="ExternalOutput").ap()

    with ExitStack() as ctx:
        wraw = ctx.enter_context(nc.sbuf_tensor([2 * C, NW], F32))
        xbuf = ctx.enter_context(nc.sbuf_tensor([2 * C, LEN], F16))
        mneg = ctx.enter_context(nc.sbuf_tensor([2 * C, NW], F16))
        weff = ctx.enter_context(nc.sbuf_tensor([2 * C, NW], F16))
        v0 = ctx.enter_context(nc.sbuf_tensor([COUT, NOUT], F32))
        wdum = ctx.enter_context(nc.sbuf_tensor([2 * C, COUT], F16))
        mdum = ctx.enter_context(nc.sbuf_tensor([2 * C, 512], F16))
        ps = ctx.enter_context(nc.psum_tensor([COUT, NOUT], F32))
        psd = ctx.enter_context(nc.psum_tensor([COUT, 512], F32))
        s_wt = ctx.enter_context(nc.semaphore())
        s_wb = ctx.enter_context(nc.semaphore())
        s_xt = ctx.enter_context(nc.semaphore())
        s_xb = ctx.enter_context(nc.semaphore())
        s_dve = ctx.enter_context(nc.semaphore())
        s_mm = ctx.enter_context(nc.semaphore())
        s_clip = ctx.enter_context(nc.semaphore())

        AL = mybir.AluOpType

        # ---- input DMAs: w halves on the two HWDGE queues, x via
        # SWDGE cast-DMA (f32 -> f16), top half first ----
        nc.sync.dma_start(wraw[0:C, :], xwin[0:C, LEN:NIN]).then_inc(s_wt, 16)
        nc.scalar.dma_start(wraw[C:2 * C, 0:NWB], xwin[C:2 * C, LEN:LEN + NWB]).then_inc(s_wb, 16)
        nc.gpsimd.dma_start(xbuf[0:C, :], xwin[0:C, 0:LEN]).then_inc(s_xt, 16)
        nc.gpsimd.dma_start(xbuf[C:2 * C, :], xwin[C:2 * C, 0:LEN]).then_inc(s_xb, 16)

        # ---- DVE weight chain: singles chunk (top-only) first ----
        nc.vector.wait_ge(s_wt, 16)
        nc.vector.tensor_scalar(mneg[0:C, NWB:NW], wraw[0:C, NWB:NW], THR, -8.0, AL.is_lt, AL.mult)
        nc.vector.scalar_tensor_tensor(weff[0:C, NWB:NW], wraw[0:C, NWB:NW], 1.0, mneg[0:C, NWB:NW], AL.mult, AL.add).then_inc(s_dve, 1)
        nc.vector.wait_ge(s_wb, 16)
        nc.vector.tensor_scalar(mneg[:, 0:NWB], wraw[:, 0:NWB], THR, -8.0, AL.is_lt, AL.mult)
        nc.vector.scalar_tensor_tensor(weff[:, 0:NWB], wraw[:, 0:NWB], 1.0, mneg[:, 0:NWB], AL.mult, AL.add).then_inc(s_dve, 1)

        # ---- PE: warm-up dummies, then 6 real matmul slots ----
        NDUM = 8
        for i in range(NDUM):
            nc.tensor.matmul(psd[:], wdum[:, 0:COUT], mdum[:],
                             start=(i == 0), stop=(i == NDUM - 1))

        nc.tensor.wait_ge(s_dve, 1)
        nc.tensor.wait_ge(s_xt, 16)
        for i, o in enumerate((2, 68, 70)):      # taps 2, 6, 8
            nc.tensor.matmul(ps[:], weff[0:C, (3 + i) * COUT:(4 + i) * COUT],
                             xbuf[0:C, o:o + NOUT], start=(i == 0), stop=False)
        nc.tensor.wait_ge(s_dve, 2)
        nc.tensor.wait_ge(s_xb, 16)
        for i, o in enumerate((0, 1, 34)):       # pairs (0,4), (1,5), (3,7)
            mm = nc.tensor.matmul(ps[:], weff[:, i * COUT:(i + 1) * COUT],
                                  xbuf[:, o:o + NOUT], start=False, stop=(i == 2))
        mm.then_inc(s_mm, 1)

        # ---- DVE post: clip straight out of PSUM, in column halves;
        # out-DMA per half on SP/ACT (fini drains cover completion) ----
        nc.vector.wait_ge(s_mm, 1)
        nc.vector.tensor_scalar(v0[:, 0:NH], ps[:, 0:NH], AMAX, AMIN, AL.min, AL.max).then_inc(s_clip, 1)
        nc.vector.tensor_scalar(v0[:, NH:NOUT], ps[:, NH:NOUT], AMAX, AMIN, AL.min, AL.max).then_inc(s_clip, 1)
        nc.sync.wait_ge(s_clip, 1)
        nc.sync.dma_start(yout[:, 0:NH], v0[:, 0:NH]).then_inc(s_wt, 16)
        nc.scalar.wait_ge(s_clip, 2)
        nc.scalar.dma_start(yout[:, NH:NOUT], v0[:, NH:NOUT]).then_inc(s_wb, 16)

    # Strip the framework const-AP memsets and the post-init all-engine
    # barrier (they are unused here; HW semaphores are zero at NEFF load
    # and re-zeroed by the NEFF epilogue). Only the construction-time
    # preamble prefix is touched.
    insts = main.instructions
    pre = [
        ins for ins in insts[:n_preamble]
        if type(ins).__name__ not in (
            "InstMemset", "InstDrain", "InstEventSemaphore", "InstRegisterMove")
    ]
    main.instructions = pre + insts[n_preamble:]

    return nc


def _get_nc():
    global _CACHED
    if _CACHED is None:
        _CACHED = _build()
    return _CACHED


def _shard_inputs(x, weight):
    xpad = np.pad(np.ascontiguousarray(x, dtype=np.float32),
                  ((0, 0), (0, 0), (1, 1), (1, 1)))
    wre = np.asarray(weight, dtype=np.float32).transpose(1, 2, 3, 0)  # [ci, kh, kw, co]
    wtap = [wre[:, t // 3, t % 3, :] for t in range(9)]               # each [C, COUT]
    w_top = np.concatenate([wtap[0], wtap[1], wtap[3], wtap[2], wtap[6], wtap[8]], axis=1)
    w_bot = np.concatenate([wtap[4], wtap[5], wtap[7]], axis=1)
    in_maps = []
    for c in range(8):
        b, q = divmod(c, 4)
        top = xpad[b, :, RPC * q:RPC * q + SECR, :].reshape(C, LEN)
        xw = np.zeros((2 * C, NIN), np.float32)
        xw[0:C, 0:LEN] = top
        xw[C:2 * C, 0:LEN - 35] = top[:, 35:]
        xw[0:C, LEN:NIN] = w_top
        xw[C:2 * C, LEN:LEN + NWB] = w_bot
        in_maps.append({"xw": np.ascontiguousarray(xw)})
    return in_maps


def kernel(x, weight):
    nc = _get_nc()
    in_maps = _shard_inputs(x, weight)
    res = run_bass_kernel_spmd(nc, in_maps, core_ids=list(range(8)))
    out = np.empty((B, COUT, H, W), dtype=np.float32)
    for c in range(8):
        b, q = divmod(c, 4)
        y = res.results[c]["y"]
        for r in range(RPC):
            out[b, :, RPC * q + r, :] = y[:, r * SECW:r * SECW + W]
    return out


# revision 17
# speedup vs baseline: 1.1293x; 1.0118x over previous
"""Trainium2 Bass kernel for nn_Conv2d_mvm (crossbar-quantized 3x3 conv).

The reference simulates a bit-sliced crossbar. Reductions:

1. The ADC clip [0, 511] can never bind (max per-xbar analog sum is
   128 rows * max slice digit 3 = 384), so the computation is exactly
   linear in the bit decompositions.

2. The weight reconstruction applies slice_w[0] = -2^14 to the whole
   MSB 2-bit digit, which is NOT true 2's complement: net effect the
   conv uses effective weights  w_eff = wi - 32768*[wi < 0]  with
   wi = rne(4096*w), and xi = rne(4096*x) exactly.

3. Precision slack (gate is rel_err < 2e-2; measured ~1.4e-3): skip
   integer rounding and fold the final /2^24 into the quant scales --
   fp16 is scale-invariant under powers of two, so
     xbuf = fp16(x)            (== fp16(4096 x) / 4096 exactly)
     weff = fp16(w - 8*[w < -1/8192])
   makes PSUM hold the final pre-clip value directly: no post scale
   pass; the DVE clips straight out of PSUM. fp16(x) is produced by
   the SWDGE cast-during-DMA, so the ACT engine runs no compute at
   all (no activation-table load on the critical path).

Implementation (8 cores, data-parallel over batch x row-blocks):
  - core c handles batch c//4, output rows 8*(c%4) .. 8*(c%4)+8.
  - K=128 tap packing: x lands twice -- partitions 0-63 the padded
    [10, 34] section flat, partitions 64-127 the same flat buffer
    shifted 35 columns (one row + one col). Taps with OFFS delta 35
    share one K=128 matmul (top half hits offset o, bottom o+35):
    pairs (0,4), (1,5), (3,7); taps 2, 6, 8 stay top-half K=64.
    6 matmul slots instead of 9, singles first (they only need the
    top x half + top-half weights, which land earliest).
    (A 5-slot variant with taps 6/7 as concurrent (0,0)/(64,0)
    row-tiles compiled but died at runtime -- base-partition-64
    K=64 matmuls are not usable.)
  - weights [128, 384] f32: col blocks 0-2 hold the pairs stacked
    top/bottom, blocks 3-5 taps 2, 6, 8 in the top half only.
  - DMA split: SP carries w_top [64,384], ACT w_bot [64,192] (both
    HWDGE, issued immediately), GpSimd the two x cast-DMAs
    (f32 DRAM -> f16 SBUF, top then bottom). Separate semaphores per
    piece so each consumer waits only for what it reads.
  - DVE: mneg = -8*[w < -1/8192] (fp32 w -- the threshold must be
    evaluated pre-fp16 or ~5 mask bits flip), weff = fp16(w + mneg),
    singles chunk first (top w only), pairs chunk second; after the
    matmuls clip v0 = min(max(psum, -8), 32767/4096) read directly
    from PSUM, in column halves so the out-DMAs start per half on
    SP/ACT (fini drains cover completion).
  - PE warm-up: 8 garbage-input K=128 N=512 matmuls issue as the
    PE's first instructions (~3.4us at the cold 1.2GHz clock), so the
    HAM activity window un-throttles the PE clock to 2.4GHz right as
    the real matmuls (which start ~3.3us after main-block entry)
    begin. They read never-written SBUF into a scratch PSUM bank --
    no memsets or semaphores needed.

All value arithmetic happens on device; the host only pads, shards,
reshapes and gathers.
"""

from contextlib import ExitStack

import numpy as np

import concourse.bass as bass
import concourse.mybir as mybir
from concourse.bass_utils import run_bass_kernel_spmd

# fixed problem shape
B, C, H, W = 2, 64, 32, 32
COUT = 64
RPC = 8                    # output rows per core
SECR = RPC + 2             # padded rows per section
SECW = W + 2               # padded width
LEN = SECR * SECW          # 340
NOUT = (RPC - 1) * SECW + W  # 270 psum columns covering all valid pixels
NW = 6 * COUT              # 384 packed weight cols (3 pairs + 3 singles)
NWB = 3 * COUT             # 192 bottom-half weight cols (pairs only)
NIN = LEN + NW             # 724 packed input columns per partition
NH = 136                   # out-DMA column split

AMAX = 32767.0 / 4096.0
AMIN = -8.0
THR = -1.0 / 8192.0        # wi = rne(4096 w) < 0  <=>  w < -1/8192

F32 = mybir.dt.float32
F16 = mybir.dt.float16

# The NEFF fini block resets every HW semaphore below the compiler's
# max-sem-num bound. Packing bass's kernel semaphores just above
# walrus's internal ones and telling walrus the bound keeps the
# program's semaphore footprint minimal.
MAX_SEM = 64


def _patch_sem_budget():
    import concourse.bass_utils as bu
    if getattr(bu, "_sem_budget_patched", False):
        return
    bass.get_walrus_max_sem_num = lambda: MAX_SEM - 8
    orig = bu.get_walrus_args

    def patched(*a, **k):
        return [*orig(*a, **k), f"--max-sem-num={MAX_SEM}"]

    bu.get_walrus_args = patched
    bu._sem_budget_patched = True


_CACHED = None


def _build():
    _patch_sem_budget()
    nc = bass.Bass("TRN2", target_bir_lowering=False, debug=False, num_devices=8,
                   monotonic_sem_count=0)
    main = nc.m.functions[0].blocks[0]
    assert main.name == "main"
    n_preamble = len(main.instructions)

    xwin = nc.dram_tensor("xw", [2 * C, NIN], F32, kind="ExternalInput").ap()
    yout = nc.dram_tensor("y", [COUT, NOUT], F32, kind="ExternalOutput").ap()

    with ExitStack() as ctx:
        wraw = ctx.enter_context(nc.sbuf_tensor([2 * C, NW], F32))
        xbuf = ctx.enter_context(nc.sbuf_tensor([2 * C, LEN], F16))
        mneg = ctx.enter_context(nc.sbuf_tensor([2 * C, NW], F16))
        weff = ctx.enter_context(nc.sbuf_tensor([2 * C, NW], F16))
        v0 = ctx.enter_context(nc.sbuf_tensor([COUT, NOUT], F32))
        wdum = ctx.enter_context(nc.sbuf_tensor([2 * C, COUT], F16))
        mdum = ctx.enter_context(nc.sbuf_tensor([2 * C, 512], F16))
        ps = ctx.enter_context(nc.psum_tensor([COUT, NOUT], F32))
        psd = ctx.enter_context(nc.psum_tensor([COUT, 512], F32))
        s_wt = ctx.enter_context(nc.semaphore())
        s_wb = ctx.enter_context(nc.semaphore())
        s_xt = ctx.enter_context(nc.semaphore())
        s_xb = ctx.enter_context(nc.semaphore())
        s_dve = ctx.enter_context(nc.semaphore())
        s_mm = ctx.enter_context(nc.semaphore())
        s_clip = ctx.enter_context(nc.semaphore())

        AL = mybir.AluOpType

        # ---- input DMAs: w halves on the two HWDGE queues, x via
        # SWDGE cast-DMA (f32 -> f16), top half first ----
        nc.sync.dma_start(wraw[0:C, :], xwin[0:C, LEN:NIN]).then_inc(s_wt, 16)
        nc.scalar.dma_start(wraw[C:2 * C, 0:NWB], xwin[C:2 * C, LEN:LEN + NWB]).then_inc(s_wb, 16)
        nc.gpsimd.dma_start(xbuf[0:C, :], xwin[0:C, 0:LEN]).then_inc(s_xt, 16)
        nc.gpsimd.dma_start(xbuf[C:2 * C, :], xwin[C:2 * C, 0:LEN]).then_inc(s_xb, 16)

        # ---- DVE weight chain: singles chunk (top-only) first ----
        nc.vector.wait_ge(s_wt, 16)
        nc.vector.tensor_scalar(mneg[0:C, NWB:NW], wraw[0:C, NWB:NW], THR, -8.0, AL.is_lt, AL.mult)
        nc.vector.scalar_tensor_tensor(weff[0:C, NWB:NW], wraw[0:C, NWB:NW], 1.0, mneg[0:C, NWB:NW], AL.mult, AL.add).then_inc(s_dve, 1)
        nc.vector.wait_ge(s_wb, 16)
        nc.vector.tensor_scalar(mneg[:, 0:NWB], wraw[:, 0:NWB], THR, -8.0, AL.is_lt, AL.mult)
        nc.vector.scalar_tensor_tensor(weff[:, 0:NWB], wraw[:, 0:NWB], 1.0, mneg[:, 0:NWB], AL.mult, AL.add).then_inc(s_dve, 1)

        # ---- PE: warm-up dummies, then 6 real matmul slots ----
        NDUM = 7
        for i in range(NDUM):
            nc.tensor.matmul(psd[:], wdum[:, 0:COUT], mdum[:],
                             start=(i == 0), stop=(i == NDUM - 1))

        nc.tensor.wait_ge(s_dve, 1)
        nc.tensor.wait_ge(s_xt, 16)
        for i, o in enumerate((2, 68, 70)):      # taps 2, 6, 8
            nc.tensor.matmul(ps[:], weff[0:C, (3 + i) * COUT:(4 + i) * COUT],
                             xbuf[0:C, o:o + NOUT], start=(i == 0), stop=False)
        nc.tensor.wait_ge(s_dve, 2)
        nc.tensor.wait_ge(s_xb, 16)
        for i, o in enumerate((0, 1, 34)):       # pairs (0,4), (1,5), (3,7)
            mm = nc.tensor.matmul(ps[:], weff[:, i * COUT:(i + 1) * COUT],
                                  xbuf[:, o:o + NOUT], start=False, stop=(i == 2))
        mm.then_inc(s_mm, 1)

        # ---- DVE post: clip straight out of PSUM, in column halves;
        # out-DMA per half on SP/ACT (fini drains cover completion) ----
        nc.vector.wait_ge(s_mm, 1)
        nc.vector.tensor_scalar(v0[:, 0:NH], ps[:, 0:NH], AMAX, AMIN, AL.min, AL.max).then_inc(s_clip, 1)
        nc.vector.tensor_scalar(v0[:, NH:NOUT], ps[:, NH:NOUT], AMAX, AMIN, AL.min, AL.max).then_inc(s_clip, 1)
        nc.sync.wait_ge(s_clip, 1)
        nc.sync.dma_start(yout[:, 0:NH], v0[:, 0:NH]).then_inc(s_wt, 16)
        nc.scalar.wait_ge(s_clip, 2)
        nc.scalar.dma_start(yout[:, NH:NOUT], v0[:, NH:NOUT]).then_inc(s_wb, 16)

    # Strip the framework const-AP memsets and the post-init all-engine
    # barrier (they are unused here; HW semaphores are zero at NEFF load
    # and re-zeroed by the NEFF epilogue). Only the construction-time
    # preamble prefix is touched.
    insts = main.instructions
    pre = [
        ins for ins in insts[:n_preamble]
        if type(ins).__name__ not in (
            "InstMemset", "InstDrain", "InstEventSemaphore", "InstRegisterMove")
    ]
    main.instructions = pre + insts[n_preamble:]

    return nc


def _get_nc():
    global _CACHED
    if _CACHED is None:
        _CACHED = _build()
    return _CACHED


def _shard_inputs(x, weight):
    xpad = np.pad(np.ascontiguousarray(x, dtype=np.float32),
                  ((0, 0), (0, 0), (1, 1), (1, 1)))
    wre = np.asarray(weight, dtype=np.float32).transpose(1, 2, 3, 0)  # [ci, kh, kw, co]
    wtap = [wre[:, t // 3, t % 3, :] for t in range(9)]               # each [C, COUT]
    w_top = np.concatenate([wtap[0], wtap[1], wtap[3], wtap[2], wtap[6], wtap[8]], axis=1)
    w_bot = np.concatenate([wtap[4], wtap[5], wtap[7]], axis=1)
    in_maps = []
    for c in range(8):
        b, q = divmod(c, 4)
        top = xpad[b, :, RPC * q:RPC * q + SECR, :].reshape(C, LEN)
        xw = np.zeros((2 * C, NIN), np.float32)
        xw[0:C, 0:LEN] = top
        xw[C:2 * C, 0:LEN - 35] = top[:, 35:]
        xw[0:C, LEN:NIN] = w_top
        xw[C:2 * C, LEN:LEN + NWB] = w_bot
        in_maps.append({"xw": np.ascontiguousarray(xw)})
    return in_maps


def kernel(x, weight):
    nc = _get_nc()
    in_maps = _shard_inputs(x, weight)
    res = run_bass_kernel_spmd(nc, in_maps, core_ids=list(range(8)))
    out = np.empty((B, COUT, H, W), dtype=np.float32)
    for c in range(8):
        b, q = divmod(c, 4)
        y = res.results[c]["y"]
        for r in range(RPC):
            out[b, :, RPC * q + r, :] = y[:, r * SECW:r * SECW + W]
    return out
